# revision 12
# baseline (speedup 1.0000x reference)
"""LoRA-MoE Linear kernel for Trainium2, 8-core SPMD.

Strategy (token-parallel):
  - Shard the 8192 tokens across 8 cores (1024 each). Host pre-transposes
    x shards and the dense weight so every matmul operand arrives with the
    contraction dim on SBUF partitions (no on-chip transposes of big data).
  - Per core: gate MLP -> top-2 routing -> local expert counts -> tiny
    AllReduce (global capacity rule) -> combine weights; LoRA down-proj
    (tmp^T = A_cat^T x) computed in [er, tok] layout; main matmul runs
    W-stationary / x-moving in float32r (full PE rate) producing out^T,
    with the LoRA up-proj matmul fused into the same PSUM accumulation.
  - Host gathers per-core out^T shards and transposes back.
"""

import numpy as np

import concourse.bacc as bacc
import concourse.bass as bass
import concourse.mybir as mybir
import concourse.tile as tile
from concourse.bass_utils import run_bass_kernel_spmd
from concourse.masks import make_identity

F32 = mybir.dt.float32
F32R = mybir.dt.float32r
AX = mybir.AxisListType
ALU = mybir.AluOpType
ACT = mybir.ActivationFunctionType

B, S, IN, OUT = 4, 2048, 4096, 4096
E, K, R = 8, 2, 16
CAP_FACTOR = 3.0
ALPHA = 1.0 / R
LN_EPS = 1e-5
N_CORES = 8
N_TOK = B * S               # 8192
TPC = N_TOK // N_CORES      # 1024 tokens per core
G4E = 4 * E                 # 32 gate hidden
ER = E * R                  # 128
KT = IN // 128              # 32 contraction tiles
NEG = -1.0e30
CAPACITY = float(int(CAP_FACTOR * N_TOK / E))  # 3072


def _r(ap):
    """View an fp32 AP as float32r for full-rate PE matmuls."""
    return ap.bitcast(F32R)


def build_bass():
    nc = bacc.Bacc(
        "TRN2", target_bir_lowering=False, debug=False, num_devices=N_CORES
    )
    xT = nc.dram_tensor("xT", [IN, TPC], F32, kind="ExternalInput")
    wT = nc.dram_tensor("wT", [IN, OUT], F32, kind="ExternalInput")
    a_cat = nc.dram_tensor("a_cat", [IN, ER], F32, kind="ExternalInput")
    b_cat = nc.dram_tensor("b_cat", [ER, OUT], F32, kind="ExternalInput")
    g1T = nc.dram_tensor("g1T", [IN, G4E], F32, kind="ExternalInput")
    g2T = nc.dram_tensor("g2T", [G4E, E], F32, kind="ExternalInput")
    gb1r = nc.dram_tensor("gb1r", [128, G4E], F32, kind="ExternalInput")
    gamr = nc.dram_tensor("gamr", [128, G4E], F32, kind="ExternalInput")
    betr = nc.dram_tensor("betr", [128, G4E], F32, kind="ExternalInput")
    gb2r = nc.dram_tensor("gb2r", [128, E], F32, kind="ExternalInput")
    repm = nc.dram_tensor("repm", [E, ER], F32, kind="ExternalInput")
    outT = nc.dram_tensor("outT", [OUT, TPC], F32, kind="ExternalOutput")

    TB = TPC // 128  # 8 token blocks of 128
    TH = TPC // 512  # 2 token halves of 512

    with tile.TileContext(nc) as tc:
        with (
            tc.tile_pool(name="big", bufs=1) as big,
            tc.tile_pool(name="consts", bufs=1) as consts,
            tc.tile_pool(name="gate", bufs=2) as gp,
            tc.tile_pool(name="route", bufs=TB) as rp,
            tc.tile_pool(name="psum_s", bufs=2, space="PSUM") as pss,
            tc.tile_pool(name="psum_c", bufs=1, space="PSUM") as psc,
            tc.tile_pool(name="psum_m", bufs=4, space="PSUM") as psm,
            tc.tile_pool(name="wslab", bufs=2) as wsp,
            tc.tile_pool(name="outp", bufs=2) as op_,
            tc.tile_pool(name="dram", bufs=1, space="DRAM") as dp,
        ):
            # ---- resident loads -------------------------------------------
            xT_sb = big.tile([128, KT, TPC], F32R)
            nc.sync.dma_start(
                xT_sb, xT.ap().bitcast(F32R).rearrange("(k p) t -> p k t", p=128)
            )
            b_sb = big.tile([128, OUT], F32R)
            nc.sync.dma_start(b_sb, b_cat.ap().bitcast(F32R))
            g1T_sb = consts.tile([128, KT, G4E], F32R)
            nc.sync.dma_start(
                g1T_sb, g1T.ap().bitcast(F32R).rearrange("(k p) g -> p k g", p=128)
            )
            g2T_sb = consts.tile([G4E, E], F32R)
            nc.sync.dma_start(g2T_sb, g2T.ap().bitcast(F32R))
            gb1_sb = consts.tile([128, G4E], F32)
            nc.sync.dma_start(gb1_sb, gb1r.ap())
            gam_sb = consts.tile([128, G4E], F32)
            nc.sync.dma_start(gam_sb, gamr.ap())
            bet_sb = consts.tile([128, G4E], F32)
            nc.sync.dma_start(bet_sb, betr.ap())
            gb2_sb = consts.tile([128, E], F32)
            nc.sync.dma_start(gb2_sb, gb2r.ap())
            rep_sb = consts.tile([E, ER], F32)
            nc.sync.dma_start(rep_sb, repm.ap())
            ident = consts.tile([128, 128], F32)
            make_identity(nc, ident)
            ones = consts.tile([128, 128], F32)
            nc.vector.memset(ones, 1.0)
            eps_sb = consts.tile([128, 1], F32)
            nc.vector.memset(eps_sb, LN_EPS)

            # ---- gate MLP + routing ---------------------------------------
            hT_sb = consts.tile([G4E, TPC], F32R)
            w_blk = []   # per-token-block combine pieces
            cnt_ps0 = psc.tile([1, E], F32)
            cnt_ps1 = psc.tile([1, E], F32)
            for tb in range(TB):
                ph = pss.tile([128, G4E], F32, tag="sm")
                for k in range(KT):
                    nc.tensor.matmul(
                        ph,
                        xT_sb[:, k, tb * 128 : (tb + 1) * 128],
                        g1T_sb[:, k],
                        start=(k == 0),
                        stop=(k == KT - 1),
                    )
                h = gp.tile([128, G4E], F32, tag="h")
                nc.vector.tensor_tensor(out=h, in0=ph, in1=gb1_sb, op=ALU.add)
                mu = gp.tile([128, 1], F32, tag="mu")
                nc.vector.tensor_reduce(out=mu, in_=h, axis=AX.X, op=ALU.add)
                nc.vector.tensor_scalar_mul(mu, mu, 1.0 / G4E)
                d = gp.tile([128, G4E], F32, tag="d")
                nc.vector.tensor_scalar_sub(d, h, mu)
                sq = gp.tile([128, G4E], F32, tag="sq")
                nc.vector.tensor_tensor(out=sq, in0=d, in1=d, op=ALU.mult)
                var = gp.tile([128, 1], F32, tag="var")
                nc.vector.tensor_reduce(out=var, in_=sq, axis=AX.X, op=ALU.add)
                std = gp.tile([128, 1], F32, tag="std")
                nc.scalar.activation(
                    std, var, ACT.Sqrt, bias=eps_sb[:, :], scale=1.0 / G4E
                )
                rstd = gp.tile([128, 1], F32, tag="rstd")
                nc.vector.reciprocal(rstd, std)
                hn = gp.tile([128, G4E], F32, tag="hn")
                nc.vector.tensor_scalar_mul(hn, d, rstd)
                nc.vector.tensor_tensor(out=hn, in0=hn, in1=gam_sb, op=ALU.mult)
                nc.vector.tensor_tensor(out=hn, in0=hn, in1=bet_sb, op=ALU.add)
                nc.vector.tensor_scalar_max(hn, hn, 0.0)
                # transpose h block -> [32, 128]
                pt = pss.tile([G4E, 128], F32, tag="sm")
                nc.tensor.transpose(pt, hn, ident)
                nc.vector.tensor_copy(hT_sb[:, tb * 128 : (tb + 1) * 128], pt)
                # gates for this block: [128 tok, 8]
                pg = pss.tile([128, E], F32, tag="sm")
                nc.tensor.matmul(
                    pg,
                    hT_sb[:, tb * 128 : (tb + 1) * 128],
                    g2T_sb,
                    start=True,
                    stop=True,
                )
                gates = rp.tile([128, E], F32, tag="gates")
                nc.vector.tensor_tensor(out=gates, in0=pg, in1=gb2_sb, op=ALU.add)
                v1 = rp.tile([128, 1], F32, tag="v1")
                nc.vector.tensor_reduce(out=v1, in_=gates, axis=AX.X, op=ALU.max)
                oh1 = rp.tile([128, E], F32, tag="oh1")
                nc.vector.tensor_scalar(
                    out=oh1, in0=gates, scalar1=v1, scalar2=None, op0=ALU.is_ge
                )
                msk = rp.tile([128, E], F32, tag="msk")
                nc.vector.tensor_scalar_mul(msk, oh1, NEG)
                nc.vector.tensor_tensor(out=msk, in0=msk, in1=gates, op=ALU.add)
                v2 = rp.tile([128, 1], F32, tag="v2")
                nc.vector.tensor_reduce(out=v2, in_=msk, axis=AX.X, op=ALU.max)
                oh2 = rp.tile([128, E], F32, tag="oh2")
                nc.vector.tensor_scalar(
                    out=oh2, in0=msk, scalar1=v2, scalar2=None, op0=ALU.is_ge
                )
                d12 = rp.tile([128, 1], F32, tag="d12")
                nc.vector.tensor_tensor(out=d12, in0=v1, in1=v2, op=ALU.subtract)
                s1 = rp.tile([128, 1], F32, tag="s1")
                nc.scalar.activation(s1, d12, ACT.Sigmoid)
                s2 = rp.tile([128, 1], F32, tag="s2")
                nc.vector.tensor_scalar(
                    out=s2, in0=s1, scalar1=-1.0, scalar2=1.0, op0=ALU.mult, op1=ALU.add
                )
                w_blk.append((oh1, oh2, s1, s2))
                # expert counts per slot (sum over tokens via ones-matmul)
                nc.tensor.matmul(
                    cnt_ps0, ones[:, 0:1], oh1,
                    start=(tb == 0), stop=(tb == TB - 1),
                )
                nc.tensor.matmul(
                    cnt_ps1, ones[:, 0:1], oh2,
                    start=(tb == 0), stop=(tb == TB - 1),
                )

            # ---- global capacity rule (AllReduce of counts) ---------------
            cnt_sb = consts.tile([1, 2 * E], F32)
            nc.vector.tensor_copy(cnt_sb[:, 0:E], cnt_ps0)
            nc.vector.tensor_copy(cnt_sb[:, E : 2 * E], cnt_ps1)
            cc_in = dp.tile([1, 2 * E], F32)
            cc_out = dp.tile([1, 2 * E], F32)
            nc.sync.dma_start(cc_in, cnt_sb)
            nc.gpsimd.collective_compute(
                "AllReduce",
                ALU.add,
                replica_groups=[list(range(N_CORES))],
                ins=[cc_in.opt()],
                outs=[cc_out.opt()],
            )
            cntg_sb = consts.tile([1, 2 * E], F32)
            nc.sync.dma_start(cntg_sb, cc_out)
            alw1 = consts.tile([1, 2 * E], F32)
            nc.vector.tensor_scalar(
                out=alw1, in0=cntg_sb, scalar1=CAPACITY + 0.5, scalar2=None,
                op0=ALU.is_le,
            )
            pb = pss.tile([128, 2 * E], F32, tag="sm")
            nc.tensor.matmul(pb, ones[0:1, :], alw1, start=True, stop=True)
            alw = consts.tile([128, 2 * E], F32)
            nc.vector.tensor_copy(alw, pb)

            # combine weights w[tok, e], then transpose to [e, tok]
            wT_sb = consts.tile([E, TPC], F32)
            for tb in range(TB):
                oh1, oh2, s1, s2 = w_blk[tb]
                t1 = gp.tile([128, E], F32, tag="t1")
                nc.vector.tensor_tensor(out=t1, in0=oh1, in1=alw[:, 0:E], op=ALU.mult)
                nc.vector.tensor_scalar_mul(t1, t1, s1)
                t2 = gp.tile([128, E], F32, tag="t2")
                nc.vector.tensor_tensor(
                    out=t2, in0=oh2, in1=alw[:, E : 2 * E], op=ALU.mult
                )
                nc.vector.tensor_scalar_mul(t2, t2, s2)
                nc.vector.tensor_tensor(out=t1, in0=t1, in1=t2, op=ALU.add)
                ptw = pss.tile([E, 128], F32, tag="sm")
                nc.tensor.transpose(ptw, t1, ident)
                nc.vector.tensor_copy(wT_sb[:, tb * 128 : (tb + 1) * 128], ptw)

            # broadcast over rank dim: wbr[e*16+r, tok]
            wbr_sb = consts.tile([128, TPC], F32)
            for th in range(TH):
                pwb = psm.tile([128, 512], F32, tag="big")
                nc.tensor.matmul(
                    pwb, rep_sb, wT_sb[:, th * 512 : (th + 1) * 512],
                    start=True, stop=True,
                )
                nc.vector.tensor_copy(wbr_sb[:, th * 512 : (th + 1) * 512], pwb)

            # ---- LoRA down-proj: tw[er, tok] = (A_cat^T x) * wbr ----------
            tw_sb = consts.tile([128, TPC], F32R)
            ptmp = [
                psm.tile([128, 512], F32, tag="big", name=f"ptmp{_i}")
                for _i in range(TH)
            ]
            for k in range(KT):
                a_t = wsp.tile([128, ER], F32R, tag="a_t")
                nc.sync.dma_start(
                    a_t, a_cat.ap().bitcast(F32R)[k * 128 : (k + 1) * 128, :]
                )
                for th in range(TH):
                    nc.tensor.matmul(
                        ptmp[th],
                        a_t,
                        xT_sb[:, k, th * 512 : (th + 1) * 512],
                        start=(k == 0),
                        stop=(k == KT - 1),
                    )
            for th in range(TH):
                nc.vector.tensor_tensor(
                    out=tw_sb[:, th * 512 : (th + 1) * 512],
                    in0=ptmp[th],
                    in1=wbr_sb[:, th * 512 : (th + 1) * 512],
                    op=ALU.mult,
                )

            # ---- main matmul (W stationary, x moving) + fused LoRA-B ------
            for oc in range(OUT // 128):
                wsl = wsp.tile([128, KT, 128], F32R, tag="wsl")
                nc.sync.dma_start(
                    wsl,
                    wT.ap().bitcast(F32R)[:, oc * 128 : (oc + 1) * 128].rearrange(
                        "(k p) c -> p k c", p=128
                    ),
                )
                for th in range(TH):
                    po = psm.tile([128, 512], F32, tag="big")
                    for k in range(KT):
                        nc.tensor.matmul(
                            po,
                            wsl[:, k],
                            xT_sb[:, k, th * 512 : (th + 1) * 512],
                            start=(k == 0),
                            stop=False,
                        )
                    nc.tensor.matmul(
                        po,
                        b_sb[:, oc * 128 : (oc + 1) * 128],
                        tw_sb[:, th * 512 : (th + 1) * 512],
                        start=False,
                        stop=True,
                    )
                    osb = op_.tile([128, 512], F32, tag="osb")
                    nc.vector.tensor_copy(osb, po)
                    nc.sync.dma_start(
                        outT.ap()[
                            oc * 128 : (oc + 1) * 128, th * 512 : (th + 1) * 512
                        ],
                        osb,
                    )
    return nc


_CACHE = {}


def _get_nc():
    if "nc" not in _CACHE:
        nc = build_bass()
        nc.finalize()
        _CACHE["nc"] = nc
    return _CACHE["nc"]


def prep_in_maps(inputs):
    x = np.asarray(inputs["x"], dtype=np.float32)
    weight = np.asarray(inputs["weight"], dtype=np.float32)
    xf = x.reshape(N_TOK, IN)
    wT = np.ascontiguousarray(weight.T)
    a_cat = np.ascontiguousarray(
        np.asarray(inputs["lora_A"], np.float32).transpose(1, 0, 2).reshape(IN, ER)
        * ALPHA
    )
    b_cat = np.ascontiguousarray(
        np.asarray(inputs["lora_B"], np.float32).reshape(ER, OUT)
    )
    g1T = np.ascontiguousarray(np.asarray(inputs["gw1"], np.float32).T)
    g2T = np.ascontiguousarray(np.asarray(inputs["gw2"], np.float32).T)
    gb1r = np.ascontiguousarray(
        np.broadcast_to(np.asarray(inputs["gb1"], np.float32), (128, G4E))
    )
    gamr = np.ascontiguousarray(
        np.broadcast_to(np.asarray(inputs["ln_gamma"], np.float32), (128, G4E))
    )
    betr = np.ascontiguousarray(
        np.broadcast_to(np.asarray(inputs["ln_beta"], np.float32), (128, G4E))
    )
    gb2r = np.ascontiguousarray(
        np.broadcast_to(np.asarray(inputs["gb2"], np.float32), (128, E))
    )
    repm = np.zeros((E, ER), np.float32)
    for e in range(E):
        repm[e, e * R : (e + 1) * R] = 1.0

    shared = dict(
        wT=wT, a_cat=a_cat, b_cat=b_cat, g1T=g1T, g2T=g2T,
        gb1r=gb1r, gamr=gamr, betr=betr, gb2r=gb2r, repm=repm,
    )
    in_maps = []
    for c in range(N_CORES):
        xT_c = np.ascontiguousarray(xf[c * TPC : (c + 1) * TPC].T)
        in_maps.append(dict(xT=xT_c, **shared))
    return in_maps


def gather(results):
    out = np.empty((N_TOK, OUT), np.float32)
    for c in range(N_CORES):
        out[c * TPC : (c + 1) * TPC] = results[c]["outT"].T
    return out.reshape(B, S, OUT)


def kernel(**inputs):
    in_maps = prep_in_maps(inputs)
    nc = _get_nc()
    res = run_bass_kernel_spmd(nc, in_maps, core_ids=list(range(N_CORES)))
    return gather(res.results)
